# revision 1
# baseline (speedup 1.0000x reference)
"""GPT decoder (V=32000,S=1024,D=768,H=12,HID=3072,L=4,B=2) on 8 trn2 cores.

Sharding: sequence-parallel body — core c owns tokens [256c, 256c+256) of the
flattened [2048] token stream (cores 0-3 = batch 0, cores 4-7 = batch 1).
Per layer, each core computes qkv for its tokens, K/V are exchanged with an
AllGather inside each 4-core batch group, attention/FFN stay local.  The tied
lm_head runs per-core over the full vocab for the local 256 tokens.
Matmuls in bf16 with fp32 PSUM accumulation; activations/norms in fp32.
Activations are feature-major [D, tok] so the contraction dim is on partitions.
"""
import math

import ml_dtypes
import numpy as np

import concourse.bass as bass
import concourse.mybir as mybir
import concourse.tile as tile
from concourse import bacc
from concourse.bass_utils import run_bass_kernel_spmd

F32 = mybir.dt.float32
BF16 = mybir.dt.bfloat16
AF = mybir.ActivationFunctionType
ALU = mybir.AluOpType

N_CORES = 8
GROUPS = [[0, 1, 2, 3], [4, 5, 6, 7]]
V, S, D, H, HID, L, B = 32000, 1024, 768, 12, 3072, 4, 2
HD = D // H          # 64
TOK = 256            # tokens per core
NK = D // 128        # 6 feature chunks
NM_QKV = 3 * D // 128   # 18
NM_HID = HID // 128     # 24
EPS = 1e-5
VB = 500             # lm_head vocab block
NVB = V // VB        # 64

TRACE = False
LAST_RESULT = None

_NC_CACHE = None


def _ln(nc, tc, pools, x_fm, g_ap, b_ap, out_bf):
    """LayerNorm over features (partition dim) via ones-matmul reductions.

    x_fm: [128, NK, 256] f32 sbuf.  g_ap/b_ap: [128,1] per-chunk slices fn.
    out_bf: [128, NK, 256] bf16 sbuf tile to fill with gamma*x_hat+beta.
    """
    ps_stat, ps_mm, tmp, stt = pools["ps_stat"], pools["ps_mm"], pools["tmp"], pools["stt"]
    ones_bf = pools["ones_bf"]
    ones_row = pools["ones_row"]

    s1 = ps_stat.tile([1, TOK], F32, tag="lnstat")
    s2 = ps_stat.tile([1, TOK], F32, tag="lnstat")
    for k in range(NK):
        xb = tmp.tile([128, TOK], BF16, tag="lnxb")
        nc.vector.tensor_copy(xb[:], x_fm[:, k, :])
        nc.tensor.matmul(s1[:], ones_bf[:], xb[:], start=(k == 0), stop=(k == NK - 1))
        sq = tmp.tile([128, TOK], BF16, tag="lnsq")
        nc.vector.tensor_mul(sq[:], xb[:], xb[:])
        nc.tensor.matmul(s2[:], ones_bf[:], sq[:], start=(k == 0), stop=(k == NK - 1))

    mean = stt.tile([1, TOK], F32, tag="mean")
    nc.vector.tensor_scalar_mul(mean[:], s1[:], 1.0 / D)
    var = stt.tile([1, TOK], F32, tag="var")
    nc.vector.tensor_scalar_mul(var[:], s2[:], 1.0 / D)
    msq = stt.tile([1, TOK], F32, tag="msq")
    nc.vector.tensor_mul(msq[:], mean[:], mean[:])
    nc.vector.tensor_sub(var[:], var[:], msq[:])
    nc.vector.tensor_scalar_add(var[:], var[:], EPS)
    rec = stt.tile([1, TOK], F32, tag="rec")
    nc.vector.reciprocal(rec[:], var[:])
    a = stt.tile([1, TOK], F32, tag="a")
    nc.scalar.sqrt(a[:], rec[:])                      # rstd = sqrt(1/(var+eps))
    colb = stt.tile([1, TOK], F32, tag="colb")
    nc.vector.tensor_mul(colb[:], mean[:], a[:])
    nc.vector.tensor_scalar_mul(colb[:], colb[:], -1.0)  # -mean*rstd

    ba = ps_mm.tile([128, TOK], F32, tag="mm")
    nc.tensor.matmul(ba[:], ones_row[:], a[:], start=True, stop=True)
    bb = ps_mm.tile([128, TOK], F32, tag="mm")
    nc.tensor.matmul(bb[:], ones_row[:], colb[:], start=True, stop=True)

    for k in range(NK):
        t = tmp.tile([128, TOK], F32, tag="lnt")
        nc.vector.tensor_mul(t[:], x_fm[:, k, :], ba[:])
        nc.vector.tensor_add(t[:], t[:], bb[:])
        nc.scalar.activation(out_bf[:, k, :], t[:], AF.Identity,
                             bias=b_ap[k], scale=g_ap[k])


def build_nc():
    nc = bacc.Bacc("TRN2", target_bir_lowering=False, debug=False,
                   enable_asserts=True, num_devices=N_CORES)

    d_x0 = nc.dram_tensor("x0", [NK, 128, TOK], F32, kind="ExternalInput")
    d_mask = nc.dram_tensor("mask", [8, 128, TOK], F32, kind="ExternalInput")
    d_qkvw = nc.dram_tensor("qkvw", [L, NK, 128, 3 * D], BF16, kind="ExternalInput")
    d_qkvb = nc.dram_tensor("qkvb", [L, 128, NM_QKV], F32, kind="ExternalInput")
    d_projw = nc.dram_tensor("projw", [L, NK, 128, D], BF16, kind="ExternalInput")
    d_projb = nc.dram_tensor("projb", [L, 128, NK], F32, kind="ExternalInput")
    d_f1w = nc.dram_tensor("f1w", [L, NK, 128, HID], BF16, kind="ExternalInput")
    d_f1b = nc.dram_tensor("f1b", [L, 128, NM_HID], F32, kind="ExternalInput")
    d_f2w = nc.dram_tensor("f2w", [L, NM_HID, 128, D], BF16, kind="ExternalInput")
    d_f2b = nc.dram_tensor("f2b", [L, 128, NK], F32, kind="ExternalInput")
    d_n1g = nc.dram_tensor("n1g", [L, 128, NK], F32, kind="ExternalInput")
    d_n1b = nc.dram_tensor("n1b", [L, 128, NK], F32, kind="ExternalInput")
    d_n2g = nc.dram_tensor("n2g", [L, 128, NK], F32, kind="ExternalInput")
    d_n2b = nc.dram_tensor("n2b", [L, 128, NK], F32, kind="ExternalInput")
    d_fing = nc.dram_tensor("fing", [128, NK], F32, kind="ExternalInput")
    d_finb = nc.dram_tensor("finb", [128, NK], F32, kind="ExternalInput")
    d_wemb = nc.dram_tensor("wemb", [NVB, NK, 128, VB], BF16, kind="ExternalInput")
    d_out = nc.dram_tensor("logits", [TOK, V], F32, kind="ExternalOutput")

    with tile.TileContext(nc) as tc:
        from contextlib import ExitStack
        with ExitStack() as ctx:
            const = ctx.enter_context(tc.tile_pool(name="const", bufs=1))
            res = ctx.enter_context(tc.tile_pool(name="res", bufs=1))
            tmp = ctx.enter_context(tc.tile_pool(name="tmp", bufs=3))
            stt = ctx.enter_context(tc.tile_pool(name="stt", bufs=2))
            attn = ctx.enter_context(tc.tile_pool(name="attn", bufs=2))
            lmo = ctx.enter_context(tc.tile_pool(name="lmo", bufs=3))
            ps_mm = ctx.enter_context(tc.tile_pool(name="ps_mm", bufs=2, space="PSUM"))
            ps_sc = ctx.enter_context(tc.tile_pool(name="ps_sc", bufs=2, space="PSUM"))
            ps_tr = ctx.enter_context(tc.tile_pool(name="ps_tr", bufs=2, space="PSUM"))
            ps_stat = ctx.enter_context(tc.tile_pool(name="ps_stat", bufs=2, space="PSUM"))
            dram = ctx.enter_context(tc.tile_pool(name="dram", bufs=2, space="DRAM"))

            identity = const.tile([128, 128], BF16)
            from concourse.masks import make_identity
            make_identity(nc, identity[:])
            ones_bf = const.tile([128, 1], BF16)
            nc.any.memset(ones_bf[:], 1.0)
            ones_row = const.tile([1, 128], F32)
            nc.any.memset(ones_row[:], 1.0)
            ones2 = const.tile([128, 128], F32)
            nc.any.memset(ones2[:], 1.0)

            # Residual stream + mask + params, resident in SBUF
            x_fm = res.tile([128, NK, TOK], F32)
            for k in range(NK):
                nc.sync.dma_start(x_fm[:, k, :], d_x0.ap()[k])
            mask_t = res.tile([128, 8, TOK], F32)
            for t in range(8):
                nc.sync.dma_start(mask_t[:, t, :], d_mask.ap()[t])
            qkvb_a = res.tile([128, L, NM_QKV], F32)
            projb_a = res.tile([128, L, NK], F32)
            f1b_a = res.tile([128, L, NM_HID], F32)
            f2b_a = res.tile([128, L, NK], F32)
            n1g_a = res.tile([128, L, NK], F32)
            n1b_a = res.tile([128, L, NK], F32)
            n2g_a = res.tile([128, L, NK], F32)
            n2b_a = res.tile([128, L, NK], F32)
            fing_a = res.tile([128, NK], F32)
            finb_a = res.tile([128, NK], F32)
            for l in range(L):
                nc.sync.dma_start(qkvb_a[:, l, :], d_qkvb.ap()[l])
                nc.sync.dma_start(projb_a[:, l, :], d_projb.ap()[l])
                nc.sync.dma_start(f1b_a[:, l, :], d_f1b.ap()[l])
                nc.sync.dma_start(f2b_a[:, l, :], d_f2b.ap()[l])
                nc.sync.dma_start(n1g_a[:, l, :], d_n1g.ap()[l])
                nc.sync.dma_start(n1b_a[:, l, :], d_n1b.ap()[l])
                nc.sync.dma_start(n2g_a[:, l, :], d_n2g.ap()[l])
                nc.sync.dma_start(n2b_a[:, l, :], d_n2b.ap()[l])
            nc.sync.dma_start(fing_a[:], d_fing.ap())
            nc.sync.dma_start(finb_a[:], d_finb.ap())

            pools = dict(ps_stat=ps_stat, ps_mm=ps_mm, tmp=tmp, stt=stt,
                         ones_bf=ones_bf, ones_row=ones_row)

            h_bf = res.tile([128, NK, TOK], BF16)
            qkv_sb = res.tile([128, NM_QKV, TOK], BF16)
            # v_own: per head 66 cols = [onesA | v(64) | onesB]; even heads use
            # cols [1:66] (ones last -> den row 64), odd heads cols [0:65]
            # (ones first -> den row 63) so AV lands vals at the head's rows.
            v_own = res.tile([128, 2, H, 66], BF16)
            nc.any.memset(v_own[:, :, :, 0:1], 1.0)
            nc.any.memset(v_own[:, :, :, 65:66], 1.0)
            k_all = res.tile([128, NK, 4 * TOK], BF16)
            v_all = res.tile([128, 8, H * 66], BF16)
            vals_fm = res.tile([128, NK, TOK], BF16)
            h2_sb = res.tile([128, NM_HID, TOK], BF16)

            def qslice(h, qt):
                return qkv_sb[(h % 2) * 64:(h % 2) * 64 + 64, h // 2,
                              128 * qt:128 * qt + 128]

            for l in range(L):
                gs = [n1g_a[:, l, k:k + 1] for k in range(NK)]
                bs = [n1b_a[:, l, k:k + 1] for k in range(NK)]
                _ln(nc, tc, pools, x_fm, gs, bs, h_bf)

                # qkv
                with tc.tile_pool(name="wqkv", bufs=NK) as pw:
                    wk = []
                    for k in range(NK):
                        w = pw.tile([128, 3 * D], BF16, tag="w")
                        nc.sync.dma_start(w[:], d_qkvw.ap()[l, k])
                        wk.append(w)
                    for m in range(NM_QKV):
                        ps = ps_mm.tile([128, TOK], F32, tag="mm")
                        for k in range(NK):
                            nc.tensor.matmul(ps[:], wk[k][:, 128 * m:128 * (m + 1)],
                                             h_bf[:, k, :],
                                             start=(k == 0), stop=(k == NK - 1))
                        scale = 1.0 / math.sqrt(HD) if m < NK else 1.0
                        nc.scalar.activation(qkv_sb[:, m, :], ps[:], AF.Identity,
                                             bias=qkvb_a[:, l, m:m + 1], scale=scale)

                # own-chunk v -> token-major
                for h in range(H):
                    o = (h % 2) * 64
                    for t in range(2):
                        src = qkv_sb[o:o + 64, 12 + h // 2, 128 * t:128 * (t + 1)]
                        pt = ps_tr.tile([128, 64], BF16, tag="tr")
                        nc.tensor.transpose(pt[:], src,
                                            identity[o:o + 64, o:o + 64])
                        nc.vector.tensor_copy(v_own[:, t, h, 1:65], pt[:])

                # KV exchange within batch group: slots 0-5 = k chunks (256 of
                # 264 cols), slots 6-11 = v_own (2 tok-chunks x 3 blocks of
                # 4 heads x 66).
                b_in = dram.tile([12, 128, 264], BF16, tag="bin")
                b_out = dram.tile([48, 128, 264], BF16, tag="bout")
                for k in range(NK):
                    nc.sync.dma_start(b_in[k, :, 0:TOK], qkv_sb[:, NK + k, :])
                for t in range(2):
                    for j in range(3):
                        nc.sync.dma_start(b_in[6 + 3 * t + j],
                                          v_own[:, t, 4 * j:4 * (j + 1), :])
                nc.gpsimd.collective_compute(
                    "AllGather", ALU.bypass, replica_groups=GROUPS,
                    ins=[b_in.opt()], outs=[b_out.opt()])
                for c in range(4):
                    for k in range(NK):
                        nc.sync.dma_start(k_all[:, k, TOK * c:TOK * (c + 1)],
                                          b_out[12 * c + k, :, 0:TOK])
                    for t in range(2):
                        for j in range(3):
                            nc.sync.dma_start(
                                v_all[:, 2 * c + t, 264 * j:264 * (j + 1)],
                                b_out[12 * c + 6 + 3 * t + j])

                # attention: S^T per kt-chunk, exp, AV with ones-col -> den row
                for h in range(H):
                    o = (h % 2) * 64
                    kslc = slice(o, o + 64)
                    av = ps_mm.tile([128, TOK], F32, tag="mm")
                    dn = ps_stat.tile([1, TOK], F32, tag="lnstat")
                    vcol = 66 * h + 1
                    for kc in range(8):
                        st = ps_sc.tile([128, TOK], F32, tag="sc")
                        nc.tensor.matmul(
                            st[:],
                            k_all[kslc, h // 2, 128 * kc:128 * (kc + 1)],
                            qkv_sb[kslc, h // 2, :],
                            start=True, stop=True)
                        nc.vector.tensor_add(st[:], st[:], mask_t[:, kc, :])
                        pt_t = attn.tile([128, TOK], BF16, tag="ptsb")
                        nc.scalar.activation(pt_t[:], st[:], AF.Exp)
                        nc.tensor.matmul(av[o:o + 64, :],
                                         v_all[:, kc, vcol:vcol + 64],
                                         pt_t[:],
                                         start=(kc == 0), stop=(kc == 7))
                        nc.tensor.matmul(dn[:], ones_bf[:], pt_t[:],
                                         start=(kc == 0), stop=(kc == 7))
                    rden = stt.tile([1, TOK], F32, tag="rden")
                    nc.vector.reciprocal(rden[:], dn[:])
                    bc = ps_tr.tile([128, TOK], F32, tag="tr")
                    nc.tensor.matmul(bc[o:o + 64, :], ones2[0:1, 0:64],
                                     rden[:], start=True, stop=True)
                    bcs = tmp.tile([128, TOK], F32, tag="lnt")
                    nc.scalar.copy(bcs[o:o + 64, :], bc[o:o + 64, :])
                    nc.vector.tensor_mul(vals_fm[o:o + 64, h // 2, :],
                                         av[o:o + 64, :], bcs[o:o + 64, :])

                # proj + residual
                with tc.tile_pool(name="wproj", bufs=NK) as pw:
                    pk = []
                    for k in range(NK):
                        w = pw.tile([128, D], BF16, tag="w")
                        nc.sync.dma_start(w[:], d_projw.ap()[l, k])
                        pk.append(w)
                    for m in range(NK):
                        ps = ps_mm.tile([128, TOK], F32, tag="mm")
                        for k in range(NK):
                            nc.tensor.matmul(ps[:], pk[k][:, 128 * m:128 * (m + 1)],
                                             vals_fm[:, k, :],
                                             start=(k == 0), stop=(k == NK - 1))
                        t = tmp.tile([128, TOK], F32, tag="lnt")
                        nc.scalar.activation(t[:], ps[:], AF.Identity,
                                             bias=projb_a[:, l, m:m + 1])
                        nc.vector.tensor_add(x_fm[:, m, :], x_fm[:, m, :], t[:])

                # LN2 + FFN
                gs = [n2g_a[:, l, k:k + 1] for k in range(NK)]
                bs = [n2b_a[:, l, k:k + 1] for k in range(NK)]
                _ln(nc, tc, pools, x_fm, gs, bs, h_bf)

                with tc.tile_pool(name="wf1", bufs=NK) as pw:
                    wf = []
                    for k in range(NK):
                        w = pw.tile([128, HID], BF16, tag="w")
                        nc.sync.dma_start(w[:], d_f1w.ap()[l, k])
                        wf.append(w)
                    for m in range(NM_HID):
                        ps = ps_mm.tile([128, TOK], F32, tag="mm")
                        for k in range(NK):
                            nc.tensor.matmul(ps[:], wf[k][:, 128 * m:128 * (m + 1)],
                                             h_bf[:, k, :],
                                             start=(k == 0), stop=(k == NK - 1))
                        nc.scalar.activation(h2_sb[:, m, :], ps[:], AF.Gelu,
                                             bias=f1b_a[:, l, m:m + 1])

                with tc.tile_pool(name="wf2", bufs=NM_HID) as pw:
                    wf = []
                    for k in range(NM_HID):
                        w = pw.tile([128, D], BF16, tag="w")
                        nc.sync.dma_start(w[:], d_f2w.ap()[l, k])
                        wf.append(w)
                    for m in range(NK):
                        ps = ps_mm.tile([128, TOK], F32, tag="mm")
                        for k in range(NM_HID):
                            nc.tensor.matmul(ps[:], wf[k][:, 128 * m:128 * (m + 1)],
                                             h2_sb[:, k, :],
                                             start=(k == 0), stop=(k == NM_HID - 1))
                        t = tmp.tile([128, TOK], F32, tag="lnt")
                        nc.scalar.activation(t[:], ps[:], AF.Identity,
                                             bias=f2b_a[:, l, m:m + 1])
                        nc.vector.tensor_add(x_fm[:, m, :], x_fm[:, m, :], t[:])

            # final LN + lm_head
            gs = [fing_a[:, k:k + 1] for k in range(NK)]
            bs = [finb_a[:, k:k + 1] for k in range(NK)]
            _ln(nc, tc, pools, x_fm, gs, bs, h_bf)

            with tc.tile_pool(name="wlm", bufs=12) as pw:
                for b in range(NVB):
                    wvs = []
                    for k in range(NK):
                        w = pw.tile([128, VB], BF16, tag="w")
                        nc.sync.dma_start(w[:], d_wemb.ap()[b, k])
                        wvs.append(w)
                    for qt in range(2):
                        ps = ps_sc.tile([128, VB], F32, tag="sc")
                        for k in range(NK):
                            nc.tensor.matmul(ps[:],
                                             h_bf[:, k, 128 * qt:128 * (qt + 1)],
                                             wvs[k][:],
                                             start=(k == 0), stop=(k == NK - 1))
                        ot = lmo.tile([128, VB], F32, tag="ot")
                        nc.vector.tensor_copy(ot[:], ps[:])
                        nc.sync.dma_start(
                            d_out.ap()[128 * qt:128 * (qt + 1), VB * b:VB * (b + 1)],
                            ot[:])

    nc.compile()
    return nc


def _prep_inputs(W_emb, pos_emb, norm1_g, norm1_b, qkv_w, qkv_b, proj_w, proj_b,
                 norm2_g, norm2_b, ffn_w1, ffn_b1, ffn_w2, ffn_b2, fin_g, fin_b,
                 input_ids):
    bf = ml_dtypes.bfloat16
    f32 = np.float32

    def tp(a):  # [L, out, in] -> [L, NK, 128, out] bf16
        a = np.asarray(a, f32)
        out_dim = a.shape[1]
        return np.ascontiguousarray(
            a.transpose(0, 2, 1).reshape(L, NK, 128, out_dim)).astype(bf)

    def btile(a, nm):  # [L, nm*128] -> [L, 128, nm]
        return np.ascontiguousarray(
            np.asarray(a, f32).reshape(L, nm, 128).transpose(0, 2, 1))

    qkv_r = np.asarray(qkv_w, f32).reshape(L, H, 3, HD, D).transpose(0, 2, 1, 3, 4) \
        .reshape(L, 3 * D, D)
    qkv_b_r = np.asarray(qkv_b, f32).reshape(L, H, 3, HD).transpose(0, 2, 1, 3) \
        .reshape(L, 3 * D).copy()
    qkv_b_r[:, :D] *= 1.0 / math.sqrt(HD)   # q bias shares the score scale

    f2w = np.asarray(ffn_w2, f32)  # [L, D, HID]
    f2w_t = np.ascontiguousarray(
        f2w.transpose(0, 2, 1).reshape(L, NM_HID, 128, D)).astype(bf)

    W_emb = np.asarray(W_emb, f32)
    wemb_t = np.ascontiguousarray(
        W_emb.T.reshape(NK, 128, NVB, VB).transpose(2, 0, 1, 3)).astype(bf)

    ids = np.asarray(input_ids).reshape(-1).astype(np.int64)
    x0 = W_emb[ids] * math.sqrt(D)
    x0 = x0 + np.asarray(pos_emb, f32)[np.tile(np.arange(S), B)]

    common = {
        "qkvw": tp(qkv_r), "qkvb": btile(qkv_b_r, NM_QKV),
        "projw": tp(np.asarray(proj_w, f32)), "projb": btile(proj_b, NK),
        "f1w": tp(np.asarray(ffn_w1, f32)), "f1b": btile(ffn_b1, NM_HID),
        "f2w": f2w_t, "f2b": btile(ffn_b2, NK),
        "n1g": btile(norm1_g, NK), "n1b": btile(norm1_b, NK),
        "n2g": btile(norm2_g, NK), "n2b": btile(norm2_b, NK),
        "fing": np.ascontiguousarray(np.asarray(fin_g, f32).reshape(NK, 128).T),
        "finb": np.ascontiguousarray(np.asarray(fin_b, f32).reshape(NK, 128).T),
        "wemb": wemb_t,
    }

    kg = np.arange(4 * TOK)
    in_maps = []
    for c in range(N_CORES):
        xs = np.ascontiguousarray(
            x0[TOK * c:TOK * (c + 1)].T.reshape(NK, 128, TOK)).astype(f32)
        p = c % 4
        qg = p * TOK + np.arange(TOK)
        m = np.where(qg[None, :] >= kg[:, None], 0.0, -1e9).astype(f32)
        m = np.ascontiguousarray(m.reshape(8, 128, TOK))
        in_maps.append({"x0": xs, "mask": m, **common})
    return in_maps


def kernel(**inputs):
    global LAST_RESULT, _NC_CACHE
    in_maps = _prep_inputs(**inputs)
    if _NC_CACHE is None:
        _NC_CACHE = build_nc()
    res = run_bass_kernel_spmd(_NC_CACHE, in_maps, list(range(N_CORES)),
                               trace=TRACE)
    LAST_RESULT = res
    logits = np.concatenate(
        [np.asarray(res.results[c]["logits"]) for c in range(N_CORES)], axis=0)
    return logits.reshape(B, S, V).astype(np.float32)



# revision 10
# speedup vs baseline: 1.3182x; 1.3182x over previous
"""GPT decoder (V=32000,S=1024,D=768,H=12,HID=3072,L=4,B=2) on 8 trn2 cores.

Sharding: sequence-parallel body with balanced causal chunks — core c
(group g=c//4, pos p=c%4) owns global 128-token chunks {p, 7-p} of batch g
("chunk A" = p at q cols 0:128, "chunk B" = 7-p at cols 128:256).  Key
chunks live in a fixed "virtual slot" order: slot 4t+r holds rank r's
chunk t (i.e. slots 0-3 = chunks 0-3, slots 4-7 = chunks 7,6,5,4), so one
uniform SPMD program does 12 of 16 score blocks per head (A: slots 0-3,
B: slots 0-7) and host-supplied 0/1 masks zero the invisible/diagonal
parts.  Per layer K/V^T are exchanged with an AllGather inside each 4-core
batch group.  The tied lm_head is vocab-sharded: final hiddens are
all-gathered across all 8 cores, each core emits a 4000-vocab slice.
Matmuls in bf16 with fp32 PSUM accumulation; activations/norms in fp32.
Activations are feature-major [D, tok] (contraction dim on partitions).
"""
import math

import ml_dtypes
import numpy as np

import concourse.bass as bass
import concourse.mybir as mybir
import concourse.tile as tile
from concourse import bacc
from concourse.bass_utils import run_bass_kernel_spmd

F32 = mybir.dt.float32
BF16 = mybir.dt.bfloat16
AF = mybir.ActivationFunctionType
ALU = mybir.AluOpType

N_CORES = 8
GROUPS = [[0, 1, 2, 3], [4, 5, 6, 7]]
ALL_GROUP = [[0, 1, 2, 3, 4, 5, 6, 7]]
V, S, D, H, HID, L, B = 32000, 1024, 768, 12, 3072, 4, 2
HD = D // H          # 64
TOK = 256            # tokens per core (2 chunks of 128)
NK = D // 128        # 6 feature chunks
NM_HID = HID // 128  # 24
NP = H // 2          # 6 head pairs
EPS = 1e-5
VSH = V // N_CORES   # 4000 vocab rows per core
VB = 500             # lm_head vocab block
NVB = VSH // VB      # 8 blocks per core
NQT = 16             # 2048 tokens / 128

TRACE = False
LAST_RESULT = None
_NC_CACHE = None


def _ln(nc, pools, x_fm, g_ap, b_ap, out_bf):
    """LayerNorm over features (partition dim) via ones-matmul reductions.

    rstd computed as exp(-0.5*ln(var+eps)) to stay inside the ln/exp ACT
    table set (no sqrt-table thrash, no slow DVE reciprocal).
    """
    ps_sc, ps_mm, tmp, stt = (pools["ps_sc"], pools["ps_mm"],
                              pools["tmp"], pools["stt"])
    ones_bf = pools["ones_bf"]
    ones_row = pools["ones_row"]

    s1 = ps_sc.tile([1, TOK], F32, tag="sc")
    s2 = ps_sc.tile([1, TOK], F32, tag="sc")
    for k in range(NK):
        xb = tmp.tile([128, TOK], BF16, tag="lnxb")
        nc.vector.tensor_copy(xb[:], x_fm[:, k, :])
        nc.tensor.matmul(s1[:], ones_bf[:], xb[:],
                         start=(k == 0), stop=(k == NK - 1))
        sq = tmp.tile([128, TOK], BF16, tag="lnsq")
        nc.vector.tensor_mul(sq[:], xb[:], xb[:])
        nc.tensor.matmul(s2[:], ones_bf[:], sq[:],
                         start=(k == 0), stop=(k == NK - 1))

    mean = stt.tile([1, TOK], F32, tag="mean")
    nc.vector.tensor_scalar_mul(mean[:], s1[:], 1.0 / D)
    var = stt.tile([1, TOK], F32, tag="var")
    nc.vector.tensor_scalar_mul(var[:], s2[:], 1.0 / D)
    msq = stt.tile([1, TOK], F32, tag="msq")
    nc.vector.tensor_mul(msq[:], mean[:], mean[:])
    nc.vector.tensor_sub(var[:], var[:], msq[:])
    nc.vector.tensor_scalar_add(var[:], var[:], EPS)
    lnv = stt.tile([1, TOK], F32, tag="lnv")
    nc.scalar.activation(lnv[:], var[:], AF.Ln)
    a = stt.tile([1, TOK], F32, tag="a")
    nc.scalar.activation(a[:], lnv[:], AF.Exp, scale=-0.5)   # rstd
    colb = stt.tile([1, TOK], F32, tag="colb")
    nc.vector.tensor_mul(colb[:], mean[:], a[:])
    nc.vector.tensor_scalar_mul(colb[:], colb[:], -1.0)      # -mean*rstd

    ba = ps_mm.tile([128, TOK], F32, tag="mm")
    nc.tensor.matmul(ba[:], ones_row[:], a[:], start=True, stop=True)
    bb = ps_mm.tile([128, TOK], F32, tag="mm")
    nc.tensor.matmul(bb[:], ones_row[:], colb[:], start=True, stop=True)

    for k in range(NK):
        t = tmp.tile([128, TOK], F32, tag="lnt")
        nc.vector.tensor_mul(t[:], x_fm[:, k, :], ba[:])
        nc.vector.tensor_add(t[:], t[:], bb[:])
        nc.scalar.activation(out_bf[:, k, :], t[:], AF.Identity,
                             bias=b_ap[k], scale=g_ap[k])


def build_nc():
    nc = bacc.Bacc("TRN2", target_bir_lowering=False, debug=False,
                   enable_asserts=True, num_devices=N_CORES)

    d_x0 = nc.dram_tensor("x0", [128, NK, TOK], F32, kind="ExternalInput")
    d_mask = nc.dram_tensor("mask", [128, 8 * 128], BF16, kind="ExternalInput")
    d_qkvw = nc.dram_tensor("qkvw", [L, 128, NK, 3 * D], BF16, kind="ExternalInput")
    d_projw = nc.dram_tensor("projw", [L, 128, NK, D], BF16, kind="ExternalInput")
    d_f1w = nc.dram_tensor("f1w", [L, 128, NK, HID], BF16, kind="ExternalInput")
    d_f2w = nc.dram_tensor("f2w", [L, 128, NM_HID, D], BF16, kind="ExternalInput")
    # packed params [128, L*78+12]; per-layer offsets:
    #   +0 qkvb(18) +18 projb(6) +24 f1b(24) +48 f2b(6)
    #   +54 n1g(6) +60 n1b(6) +66 n2g(6) +72 n2b(6); then fing(6) finb(6)
    d_prm = nc.dram_tensor("prm", [128, L * 78 + 12], F32, kind="ExternalInput")
    d_selt = nc.dram_tensor("selt", [128, 23], BF16, kind="ExternalInput")
    d_selc = nc.dram_tensor("selc", [12, NP * 128], F32, kind="ExternalInput")
    d_wemb = nc.dram_tensor("wemb", [NVB, 128, NK, VB], BF16, kind="ExternalInput")
    d_out = nc.dram_tensor("logits", [NQT, 128, VSH], BF16, kind="ExternalOutput")

    with tile.TileContext(nc) as tc:
        from contextlib import ExitStack
        with ExitStack() as ctx:
            const = ctx.enter_context(tc.tile_pool(name="const", bufs=1))
            res = ctx.enter_context(tc.tile_pool(name="res", bufs=1))
            dram = ctx.enter_context(tc.tile_pool(name="dram", bufs=2, space="DRAM"))

            ones_bf = const.tile([128, 1], BF16)
            nc.any.memset(ones_bf[:], 1.0)
            ones_row = const.tile([1, 128], F32)
            nc.any.memset(ones_row[:], 1.0)
            # selT: col j of slice [:,11-h:23-h] is 1 iff j==h (den rows)
            selT = const.tile([128, 23], BF16)
            nc.sync.dma_start(selT[:], d_selt.ap())
            # selC: [12, NP*128] f32, col 128*hp+m -> one-hot row 2hp+(m>=64)
            selC = const.tile([12, NP * 128], F32)
            nc.sync.dma_start(selC[:], d_selc.ap())

            # Per-core data resident in SBUF
            x_fm = res.tile([128, NK, TOK], F32)
            nc.sync.dma_start(x_fm[:], d_x0.ap())
            mask_t = res.tile([128, 8 * 128], BF16)
            nc.sync.dma_start(mask_t[:], d_mask.ap())
            prm = res.tile([128, L * 78 + 12], F32)
            nc.sync.dma_start(prm[:], d_prm.ap())

            def pc(off):   # [128,1] AP at packed-param column `off`
                return prm[:, off:off + 1]

            h_bf = res.tile([128, NK, TOK], BF16)
            qkv_sb = res.tile([128, 12, TOK], BF16)   # Q pairs 0-5, K pairs 6-11
            v_stage = res.tile([128, 2, D], BF16)     # V^T per own tok chunk
            k_all = res.tile([128, NP, 8 * 128], BF16)
            v_all = res.tile([128, 8, D], BF16)       # [tok, key slot, head*64]
            vals_f = res.tile([128, NP, TOK], F32)    # unnormalized attn vals
            vals_fm = res.tile([128, NP, TOK], BF16)
            h2_sb = res.tile([128, NM_HID, TOK], BF16)
            f2part = res.tile([128, NK, TOK], F32)

            with ExitStack() as lctx:
                wpool = lctx.enter_context(tc.tile_pool(name="wpool", bufs=1))
                tmp = lctx.enter_context(tc.tile_pool(name="tmp", bufs=3))
                stt = lctx.enter_context(tc.tile_pool(name="stt", bufs=2))
                psb = lctx.enter_context(tc.tile_pool(name="psb", bufs=2))
                ps_mm = lctx.enter_context(tc.tile_pool(name="ps_mm", bufs=2, space="PSUM"))
                ps_sc = lctx.enter_context(tc.tile_pool(name="ps_sc", bufs=3, space="PSUM"))
                ps_av = lctx.enter_context(tc.tile_pool(name="ps_av", bufs=2, space="PSUM"))
                ps_dn = lctx.enter_context(tc.tile_pool(name="ps_dn", bufs=1, space="PSUM"))

                pools = dict(ps_sc=ps_sc, ps_mm=ps_mm, tmp=tmp, stt=stt,
                             ones_bf=ones_bf, ones_row=ones_row)

                for l in range(L):
                    po = 78 * l
                    _ln(nc, pools, x_fm,
                        [pc(po + 54 + k) for k in range(NK)],
                        [pc(po + 60 + k) for k in range(NK)], h_bf)

                    wq = wpool.tile([128, NK, 3 * D], BF16, tag="wqkv")
                    nc.sync.dma_start(wq[:], d_qkvw.ap()[l])

                    # K first (qkv out cols 768..1536 -> qkv_sb slots 6-11)
                    for m in range(6):
                        ps = ps_mm.tile([128, TOK], F32, tag="mm")
                        for k in range(NK):
                            nc.tensor.matmul(
                                ps[:], wq[:, k, D + 128 * m:D + 128 * (m + 1)],
                                h_bf[:, k, :], start=(k == 0), stop=(k == NK - 1))
                        nc.scalar.activation(qkv_sb[:, 6 + m, :], ps[:], AF.Identity,
                                             bias=pc(po + 6 + m))

                    # V^T: activations stationary, V weight cols streamed.
                    for t in range(2):
                        for hf in range(2):
                            ps = ps_mm.tile([128, 384], F32, tag="mm")
                            for k in range(NK):
                                nc.tensor.matmul(
                                    ps[:], h_bf[:, k, 128 * t:128 * (t + 1)],
                                    wq[:, k, 2 * D + 384 * hf:2 * D + 384 * (hf + 1)],
                                    start=(k == 0), stop=(k == NK - 1))
                            nc.scalar.copy(
                                v_stage[:, t, 384 * hf:384 * (hf + 1)], ps[:])

                    # KV exchange: slabs 0-5 = K pair-chunks, 6-11 = V^T
                    # (tok chunk t, 256-col window j) at slab 6+3t+j.
                    b_in = dram.tile([12, 128, 256], BF16, tag="bin")
                    b_out = dram.tile([4, 12, 128, 256], BF16, tag="bout")
                    for k in range(6):
                        nc.sync.dma_start(b_in[k], qkv_sb[:, 6 + k, :])
                    for t in range(2):
                        for j in range(3):
                            nc.sync.dma_start(b_in[6 + 3 * t + j],
                                              v_stage[:, t, 256 * j:256 * (j + 1)])
                    nc.gpsimd.collective_compute(
                        "AllGather", ALU.bypass, replica_groups=GROUPS,
                        ins=[b_in.opt()], outs=[b_out.opt()])

                    # Q while the AllGather flies (cols 0..768 -> slots 0-5)
                    for m in range(6):
                        ps = ps_mm.tile([128, TOK], F32, tag="mm")
                        for k in range(NK):
                            nc.tensor.matmul(
                                ps[:], wq[:, k, 128 * m:128 * (m + 1)],
                                h_bf[:, k, :], start=(k == 0), stop=(k == NK - 1))
                        nc.scalar.activation(qkv_sb[:, m, :], ps[:], AF.Identity,
                                             bias=pc(po + m))

                    # prefetch next-phase weights during the collective
                    wp = wpool.tile([128, NK, D], BF16, tag="wproj")
                    nc.sync.dma_start(wp[:], d_projw.ap()[l])

                    # unpack: key slot 4t+r <- rank r's chunk t
                    for t in range(2):
                        for r in range(4):
                            for k in range(6):
                                nc.sync.dma_start(
                                    k_all[:, k, 128 * (4 * t + r):128 * (4 * t + r + 1)],
                                    b_out[r, k, :, 128 * t:128 * (t + 1)])
                            for j in range(3):
                                nc.sync.dma_start(
                                    v_all[:, 4 * t + r, 256 * j:256 * (j + 1)],
                                    b_out[r, 6 + 3 * t + j])

                    # attention: A = q cols 0:128 (slots 0-3),
                    #            B = q cols 128:256 (slots 0-7)
                    den_all = ps_dn.tile([12, TOK], F32, tag="dn")
                    for hp in range(NP):
                        pA = [None, None]
                        pB = [None, None]
                        for h01 in range(2):
                            o = 64 * h01
                            qsl = slice(o, o + 64)
                            stA = ps_sc.tile([128, 512], F32, tag="sc")
                            for s4 in range(4):
                                nc.tensor.matmul(
                                    stA[:, 128 * s4:128 * (s4 + 1)],
                                    k_all[qsl, hp, 128 * s4:128 * (s4 + 1)],
                                    qkv_sb[qsl, hp, 0:128],
                                    start=True, stop=True)
                            pA[h01] = psb.tile([128, 512], BF16, tag=f"pA{h01}", name=f"pA_{hp}_{h01}")
                            nc.scalar.activation(pA[h01][:], stA[:], AF.Exp)
                            nc.vector.tensor_mul(pA[h01][:], pA[h01][:],
                                                 mask_t[:, 0:512])
                            stB1 = ps_sc.tile([128, 512], F32, tag="sc")
                            stB2 = ps_sc.tile([128, 512], F32, tag="sc")
                            for s8 in range(8):
                                dst = stB1 if s8 < 4 else stB2
                                nc.tensor.matmul(
                                    dst[:, 128 * (s8 % 4):128 * (s8 % 4 + 1)],
                                    k_all[qsl, hp, 128 * s8:128 * (s8 + 1)],
                                    qkv_sb[qsl, hp, 128:256],
                                    start=True, stop=True)
                            pB[h01] = psb.tile([128, 1024], BF16, tag=f"pB{h01}", name=f"pB_{hp}_{h01}")
                            nc.scalar.activation(pB[h01][:, 0:512], stB1[:], AF.Exp)
                            nc.scalar.activation(pB[h01][:, 512:1024], stB2[:], AF.Exp)
                            nc.vector.tensor_mul(pB[h01][:, 512:1024],
                                                 pB[h01][:, 512:1024],
                                                 mask_t[:, 512:1024])

                        # AV (col-tiled head pair) + denominators
                        for cab in range(2):        # 0 = A, 1 = B
                            nsl = 4 if cab == 0 else 8
                            qq = slice(128 * cab, 128 * (cab + 1))
                            av2 = ps_av.tile([128, 128], F32, tag="av")
                            for h01 in range(2):
                                hh = 2 * hp + h01
                                p_t = (pA if cab == 0 else pB)[h01]
                                for s in range(nsl):
                                    nc.tensor.matmul(
                                        av2[64 * h01:64 * (h01 + 1), :],
                                        v_all[:, s, 64 * hh:64 * (hh + 1)],
                                        p_t[:, 128 * s:128 * (s + 1)],
                                        start=(s == 0), stop=(s == nsl - 1))
                            # dens: one global accumulation group in den_all;
                            # each MM adds head hh's sums (zeros elsewhere)
                            for h01 in range(2):
                                hh = 2 * hp + h01
                                p_t = (pA if cab == 0 else pB)[h01]
                                for s in range(nsl):
                                    first = (hp == 0 and cab == 0
                                             and h01 == 0 and s == 0)
                                    last = (hp == NP - 1 and cab == 1
                                            and h01 == 1 and s == nsl - 1)
                                    nc.tensor.matmul(
                                        den_all[:, qq],
                                        selT[:, 11 - hh:23 - hh],
                                        p_t[:, 128 * s:128 * (s + 1)],
                                        start=first, stop=last)
                            nc.vector.tensor_copy(vals_f[:, hp, qq], av2[:])

                    # batched reciprocal 1/x = exp(-ln(x)), then normalize
                    lden = stt.tile([12, TOK], F32, tag="lden")
                    nc.scalar.activation(lden[:], den_all[:], AF.Ln)
                    rden12 = stt.tile([12, TOK], F32, tag="rden")
                    nc.scalar.activation(rden12[:], lden[:], AF.Exp, scale=-1.0)
                    for hp in range(NP):
                        bc = ps_mm.tile([128, TOK], F32, tag="mm")
                        nc.tensor.matmul(bc[:], selC[:, 128 * hp:128 * (hp + 1)],
                                         rden12[:], start=True, stop=True)
                        nc.vector.tensor_mul(vals_fm[:, hp, :],
                                             vals_f[:, hp, :], bc[:])
                        nc.vector.tensor_scalar_add(vals_fm[:, hp, :],
                                                    vals_fm[:, hp, :],
                                                    pc(po + 12 + hp))

                    # proj + residual
                    for m in range(NK):
                        ps = ps_mm.tile([128, TOK], F32, tag="mm")
                        for k in range(NK):
                            nc.tensor.matmul(ps[:], wp[:, k, 128 * m:128 * (m + 1)],
                                             vals_fm[:, k, :],
                                             start=(k == 0), stop=(k == NK - 1))
                        t2 = tmp.tile([128, TOK], F32, tag="lnt")
                        nc.scalar.activation(t2[:], ps[:], AF.Identity,
                                             bias=pc(po + 18 + m))
                        nc.vector.tensor_add(x_fm[:, m, :], x_fm[:, m, :], t2[:])

                    # LN2 + FFN
                    _ln(nc, pools, x_fm,
                        [pc(po + 66 + k) for k in range(NK)],
                        [pc(po + 72 + k) for k in range(NK)], h_bf)

                    for hf in range(2):
                        w1 = wpool.tile([128, NK, HID // 2], BF16, tag="wf1",
                                        name=f"w1_{l}_{hf}")
                        nc.sync.dma_start(
                            w1[:], d_f1w.ap()[l][:, :, 1536 * hf:1536 * (hf + 1)])
                        for m in range(12):
                            mm = 12 * hf + m
                            ps = ps_mm.tile([128, TOK], F32, tag="mm")
                            for k in range(NK):
                                nc.tensor.matmul(
                                    ps[:], w1[:, k, 128 * m:128 * (m + 1)],
                                    h_bf[:, k, :],
                                    start=(k == 0), stop=(k == NK - 1))
                            nc.scalar.activation(h2_sb[:, mm, :], ps[:], AF.Gelu,
                                                 bias=pc(po + 24 + mm))

                    for hf in range(2):
                        w2 = wpool.tile([128, 12, D], BF16, tag="wf2",
                                        name=f"w2_{l}_{hf}")
                        nc.sync.dma_start(
                            w2[:], d_f2w.ap()[l][:, 12 * hf:12 * (hf + 1), :])
                        for m in range(NK):
                            ps = ps_mm.tile([128, TOK], F32, tag="mm")
                            for k in range(12):
                                nc.tensor.matmul(
                                    ps[:], w2[:, k, 128 * m:128 * (m + 1)],
                                    h2_sb[:, 12 * hf + k, :],
                                    start=(k == 0), stop=(k == 11))
                            if hf == 0:
                                nc.vector.tensor_copy(f2part[:, m, :], ps[:])
                            else:
                                nc.vector.tensor_add(f2part[:, m, :],
                                                     f2part[:, m, :], ps[:])
                                t2 = tmp.tile([128, TOK], F32, tag="lnt")
                                nc.scalar.activation(t2[:], f2part[:, m, :],
                                                     AF.Identity,
                                                     bias=pc(po + 48 + m))
                                nc.vector.tensor_add(x_fm[:, m, :],
                                                     x_fm[:, m, :], t2[:])

                # final LN
                _ln(nc, pools, x_fm,
                    [pc(78 * L + k) for k in range(NK)],
                    [pc(78 * L + 6 + k) for k in range(NK)], h_bf)

            # ---- all-gather final hiddens across all 8 cores ----
            b2_in = dram.tile([NK, 128, TOK], BF16, tag="b2in")
            b2_out = dram.tile([N_CORES, NK, 128, TOK], BF16, tag="b2out")
            for k in range(NK):
                nc.sync.dma_start(b2_in[k], h_bf[:, k, :])
            nc.gpsimd.collective_compute(
                "AllGather", ALU.bypass, replica_groups=ALL_GROUP,
                ins=[b2_in.opt()], outs=[b2_out.opt()])

            with ExitStack() as mctx:
                lmw = mctx.enter_context(tc.tile_pool(name="lmw", bufs=2))
                lmo = mctx.enter_context(tc.tile_pool(name="lmo", bufs=3))
                lmh = mctx.enter_context(tc.tile_pool(name="lmh", bufs=1))
                ps_lm = mctx.enter_context(tc.tile_pool(name="ps_lm", bufs=8, space="PSUM"))

                h_all = lmh.tile([128, NK, NQT * 128], BF16)
                for r in range(N_CORES):
                    for k in range(NK):
                        nc.sync.dma_start(h_all[:, k, TOK * r:TOK * (r + 1)],
                                          b2_out[r, k])

                # lm_head: 2 groups of 4 vocab blocks; LDW amortized 4x
                for g2 in range(2):
                    wv = lmw.tile([128, NK, 4 * VB], BF16, tag="wv")
                    for j in range(4):
                        nc.sync.dma_start(wv[:, :, VB * j:VB * (j + 1)],
                                          d_wemb.ap()[4 * g2 + j])
                    for qt in range(NQT):
                        pss = [ps_lm.tile([128, VB], F32, tag="lmps",
                                          name=f"lmps_{g2}_{qt}_{j}")
                               for j in range(4)]
                        for k in range(NK):
                            lhs = h_all[:, k, 128 * qt:128 * (qt + 1)]
                            for j in range(4):
                                nc.tensor.matmul(pss[j][:], lhs,
                                                 wv[:, k, VB * j:VB * (j + 1)],
                                                 start=(k == 0), stop=(k == NK - 1))
                        ot = lmo.tile([128, 4 * VB], BF16, tag="ot")
                        nc.vector.tensor_copy(ot[:, 0:VB], pss[0][:])
                        nc.vector.tensor_copy(ot[:, VB:2 * VB], pss[1][:])
                        nc.scalar.copy(ot[:, 2 * VB:3 * VB], pss[2][:])
                        nc.scalar.copy(ot[:, 3 * VB:4 * VB], pss[3][:])
                        nc.sync.dma_start(
                            d_out.ap()[qt][:, 2000 * g2:2000 * (g2 + 1)], ot[:])

    nc.compile()
    return nc


def _prep_inputs(W_emb, pos_emb, norm1_g, norm1_b, qkv_w, qkv_b, proj_w, proj_b,
                 norm2_g, norm2_b, ffn_w1, ffn_b1, ffn_w2, ffn_b2, fin_g, fin_b,
                 input_ids):
    bf = ml_dtypes.bfloat16
    f32 = np.float32

    def tp(a):  # [L, out, in] -> [L, 128, NK, out] bf16 (partition-major)
        a = np.asarray(a, f32)
        out_dim = a.shape[1]
        return np.ascontiguousarray(
            a.transpose(0, 2, 1).reshape(L, NK, 128, out_dim)
            .transpose(0, 2, 1, 3)).astype(bf)

    # qkv reorder [L,H,3,HD,D] -> [L,3,H,HD,D]; q pre-scaled by 1/sqrt(HD)
    qkv_r = np.asarray(qkv_w, f32).reshape(L, H, 3, HD, D) \
        .transpose(0, 2, 1, 3, 4).copy()
    qkv_r[:, 0] *= 1.0 / math.sqrt(HD)
    qkv_r = qkv_r.reshape(L, 3 * D, D)
    qkv_b_r = np.asarray(qkv_b, f32).reshape(L, H, 3, HD) \
        .transpose(0, 2, 1, 3).copy()
    qkv_b_r[:, 0] *= 1.0 / math.sqrt(HD)
    qkv_b_r = qkv_b_r.reshape(L, 3 * D)

    f2w_t = np.ascontiguousarray(
        np.asarray(ffn_w2, f32).transpose(0, 2, 1)
        .reshape(L, NM_HID, 128, D).transpose(0, 2, 1, 3)).astype(bf)

    def btile(a, nm):  # [L, nm*128] -> [L, 128, nm]
        return np.asarray(a, f32).reshape(L, nm, 128).transpose(0, 2, 1)

    prm = np.zeros((128, L * 78 + 12), f32)
    packs = [(0, btile(qkv_b_r, 18)),
             (18, btile(proj_b, NK)), (24, btile(ffn_b1, NM_HID)),
             (48, btile(ffn_b2, NK)), (54, btile(norm1_g, NK)),
             (60, btile(norm1_b, NK)), (66, btile(norm2_g, NK)),
             (72, btile(norm2_b, NK))]
    for l in range(L):
        for off, arr in packs:
            prm[:, 78 * l + off:78 * l + off + arr.shape[2]] = arr[l]
    prm[:, 78 * L:78 * L + 6] = np.asarray(fin_g, f32).reshape(NK, 128).T
    prm[:, 78 * L + 6:78 * L + 12] = np.asarray(fin_b, f32).reshape(NK, 128).T

    W_emb = np.asarray(W_emb, f32)
    wemb_slices = []
    for c in range(N_CORES):
        sl = W_emb[VSH * c:VSH * (c + 1)].T  # [768, 4000]
        wemb_slices.append(np.ascontiguousarray(
            sl.reshape(NK, 128, NVB, VB).transpose(2, 1, 0, 3)).astype(bf))

    ids = np.asarray(input_ids).reshape(B, S).astype(np.int64)
    pos = np.asarray(pos_emb, f32)

    selt = np.zeros((128, 23), f32)
    selt[:, 11] = 1.0
    selc = np.zeros((12, NP * 128), f32)
    for hp in range(NP):
        selc[2 * hp, 128 * hp:128 * hp + 64] = 1.0
        selc[2 * hp + 1, 128 * hp + 64:128 * (hp + 1)] = 1.0

    common = {
        "qkvw": tp(qkv_r), "projw": tp(np.asarray(proj_w, f32)),
        "f1w": tp(np.asarray(ffn_w1, f32)), "f2w": f2w_t,
        "prm": prm, "selt": selt.astype(bf), "selc": selc,
    }

    in_maps = []
    for c in range(N_CORES):
        g, p = c // 4, c % 4
        ca, cb = p, 7 - p
        tok_idx = np.concatenate([np.arange(128 * ca, 128 * (ca + 1)),
                                  np.arange(128 * cb, 128 * (cb + 1))])
        x0 = W_emb[ids[g, tok_idx]] * math.sqrt(D) + pos[tok_idx]
        xs = np.ascontiguousarray(
            x0.T.reshape(NK, 128, TOK).transpose(1, 0, 2)).astype(f32)
        # masks: slots 0-3 (key chunks 0-3) vs chunk A; slots 4-7
        # (key chunk 7-r at slot 4+r) vs chunk B
        m8 = np.zeros((8, 128, 128), f32)
        qa_pos = 128 * ca + np.arange(128)
        qb_pos = 128 * cb + np.arange(128)
        for s4 in range(4):
            kpos = 128 * s4 + np.arange(128)
            m8[s4] = (kpos[:, None] <= qa_pos[None, :])
        for r in range(4):
            kpos = 128 * (7 - r) + np.arange(128)
            m8[4 + r] = (kpos[:, None] <= qb_pos[None, :])
        in_maps.append({"x0": xs,
                        "mask": np.ascontiguousarray(
                            m8.transpose(1, 0, 2).reshape(128, 8 * 128)).astype(bf),
                        "wemb": wemb_slices[c], **common})
    return in_maps


def kernel(**inputs):
    global LAST_RESULT, _NC_CACHE
    in_maps = _prep_inputs(**inputs)
    if _NC_CACHE is None:
        _NC_CACHE = build_nc()
    res = run_bass_kernel_spmd(_NC_CACHE, in_maps, list(range(N_CORES)),
                               trace=TRACE)
    LAST_RESULT = res
    out = np.zeros((B * S, V), np.float32)
    for c in range(N_CORES):
        o = np.asarray(res.results[c]["logits"]).astype(np.float32)
        for qt in range(NQT):
            r, t = qt // 2, qt % 2
            g, p = r // 4, r % 4
            chunk = 8 * g + (p if t == 0 else 7 - p)
            out[128 * chunk:128 * (chunk + 1), VSH * c:VSH * (c + 1)] = o[qt]
    return out.reshape(B, S, V)


# revision 14
# speedup vs baseline: 1.4163x; 1.0744x over previous
"""GPT decoder (V=32000,S=1024,D=768,H=12,HID=3072,L=4,B=2) on 8 trn2 cores.

Sharding: sequence-parallel body with balanced causal chunks — core c
(group g=c//4, pos p=c%4) owns global 128-token chunks {p, 7-p} of batch g
("chunk A" = p at q cols 0:128, "chunk B" = 7-p at cols 128:256).  Key
chunks live in a fixed "virtual slot" order: slot 4t+r holds rank r's
chunk t (i.e. slots 0-3 = chunks 0-3, slots 4-7 = chunks 7,6,5,4), so one
uniform SPMD program does 12 of 16 score blocks per head (A: slots 0-3,
B: slots 0-7) and host-supplied 0/1 masks zero the invisible/diagonal
parts.  Per layer K/V^T are exchanged with an AllGather inside each 4-core
batch group.  The tied lm_head is vocab-sharded: final hiddens are
all-gathered across all 8 cores, each core emits a 4000-vocab slice.
Matmuls in bf16 with fp32 PSUM accumulation; activations/norms in fp32.
Activations are feature-major [D, tok] (contraction dim on partitions).
"""
import math

import ml_dtypes
import numpy as np

import concourse.bass as bass
import concourse.mybir as mybir
import concourse.tile as tile
from concourse import bacc
from concourse.bass_utils import run_bass_kernel_spmd

F32 = mybir.dt.float32
BF16 = mybir.dt.bfloat16
AF = mybir.ActivationFunctionType
ALU = mybir.AluOpType

N_CORES = 8
GROUPS = [[0, 1, 2, 3], [4, 5, 6, 7]]
ALL_GROUP = [[0, 1, 2, 3, 4, 5, 6, 7]]
V, S, D, H, HID, L, B = 32000, 1024, 768, 12, 3072, 4, 2
HD = D // H          # 64
TOK = 256            # tokens per core (2 chunks of 128)
NK = D // 128        # 6 feature chunks
NM_HID = HID // 128  # 24
NP = H // 2          # 6 head pairs
EPS = 1e-5
VSH = V // N_CORES   # 4000 vocab rows per core
VB = 500             # lm_head vocab block
NVB = VSH // VB      # 8 blocks per core
NQT = 16             # 2048 tokens / 128

TRACE = False
LAST_RESULT = None
_NC_CACHE = None


def _ln(nc, pools, x_fm, g_ap, b_ap, out_bf):
    """LayerNorm over features (partition dim) via ones-matmul reductions.

    rstd computed as exp(-0.5*ln(var+eps)) to stay inside the ln/exp ACT
    table set (no sqrt-table thrash, no slow DVE reciprocal).
    """
    ps_sc, ps_mm, tmp, stt = (pools["ps_sc"], pools["ps_mm"],
                              pools["tmp"], pools["stt"])
    ones_bf = pools["ones_bf"]
    ones_row = pools["ones_row"]

    s1 = ps_sc.tile([1, TOK], F32, tag="sc")
    s2 = ps_sc.tile([1, TOK], F32, tag="sc")
    for k in range(NK):
        xb = tmp.tile([128, TOK], BF16, tag="lnxb")
        nc.vector.tensor_copy(xb[:], x_fm[:, k, :])
        nc.tensor.matmul(s1[:], ones_bf[:], xb[:],
                         start=(k == 0), stop=(k == NK - 1))
        sq = tmp.tile([128, TOK], BF16, tag="lnsq")
        nc.vector.tensor_mul(sq[:], xb[:], xb[:])
        nc.tensor.matmul(s2[:], ones_bf[:], sq[:],
                         start=(k == 0), stop=(k == NK - 1))

    mean = stt.tile([1, TOK], F32, tag="mean")
    nc.vector.tensor_scalar_mul(mean[:], s1[:], 1.0 / D)
    var = stt.tile([1, TOK], F32, tag="var")
    nc.vector.tensor_scalar_mul(var[:], s2[:], 1.0 / D)
    msq = stt.tile([1, TOK], F32, tag="msq")
    nc.vector.tensor_mul(msq[:], mean[:], mean[:])
    nc.vector.tensor_sub(var[:], var[:], msq[:])
    nc.vector.tensor_scalar_add(var[:], var[:], EPS)
    rec = stt.tile([1, TOK], F32, tag="rec")
    nc.vector.reciprocal(rec[:], var[:])
    a = stt.tile([1, TOK], F32, tag="a")
    nc.scalar.activation(a[:], rec[:], AF.Sqrt)              # rstd
    colb = stt.tile([1, TOK], F32, tag="colb")
    nc.vector.tensor_mul(colb[:], mean[:], a[:])
    nc.vector.tensor_scalar_mul(colb[:], colb[:], -1.0)      # -mean*rstd

    ba = ps_mm.tile([128, TOK], F32, tag="mm")
    nc.tensor.matmul(ba[:], ones_row[:], a[:], start=True, stop=True)
    bb = ps_mm.tile([128, TOK], F32, tag="mm")
    nc.tensor.matmul(bb[:], ones_row[:], colb[:], start=True, stop=True)

    for k in range(NK):
        t = tmp.tile([128, TOK], F32, tag="lnt")
        nc.vector.tensor_mul(t[:], x_fm[:, k, :], ba[:])
        nc.vector.tensor_add(t[:], t[:], bb[:])
        nc.scalar.activation(out_bf[:, k, :], t[:], AF.Identity,
                             bias=b_ap[k], scale=g_ap[k])


def build_nc():
    nc = bacc.Bacc("TRN2", target_bir_lowering=False, debug=False,
                   enable_asserts=True, num_devices=N_CORES)

    d_x0 = nc.dram_tensor("x0", [128, NK, TOK], F32, kind="ExternalInput")
    d_mask = nc.dram_tensor("mask", [128, 8 * 128], BF16, kind="ExternalInput")
    d_qkvw = nc.dram_tensor("qkvw", [L, 128, NK, 3 * D], BF16, kind="ExternalInput")
    d_projw = nc.dram_tensor("projw", [L, 128, NK, D], BF16, kind="ExternalInput")
    d_f1w = nc.dram_tensor("f1w", [L, 128, NK, HID], BF16, kind="ExternalInput")
    d_f2w = nc.dram_tensor("f2w", [L, 128, NM_HID, D], BF16, kind="ExternalInput")
    # packed params [128, L*78+12]; per-layer offsets:
    #   +0 qkvb(18) +18 projb(6) +24 f1b(24) +48 f2b(6)
    #   +54 n1g(6) +60 n1b(6) +66 n2g(6) +72 n2b(6); then fing(6) finb(6)
    d_prm = nc.dram_tensor("prm", [128, L * 78 + 12], F32, kind="ExternalInput")
    d_selt = nc.dram_tensor("selt", [128, 23], BF16, kind="ExternalInput")
    d_selc = nc.dram_tensor("selc", [12, NP * 128], BF16, kind="ExternalInput")
    d_wemb = nc.dram_tensor("wemb", [NVB, 128, NK, VB], BF16, kind="ExternalInput")
    d_out = nc.dram_tensor("logits", [NQT, 128, VSH], BF16, kind="ExternalOutput")

    with tile.TileContext(nc) as tc:
        from contextlib import ExitStack
        with ExitStack() as ctx:
            const = ctx.enter_context(tc.tile_pool(name="const", bufs=1))
            res = ctx.enter_context(tc.tile_pool(name="res", bufs=1))
            dram = ctx.enter_context(tc.tile_pool(name="dram", bufs=2, space="DRAM"))

            ones_bf = const.tile([128, 1], BF16)
            nc.any.memset(ones_bf[:], 1.0)
            ones_row = const.tile([1, 128], F32)
            nc.any.memset(ones_row[:], 1.0)
            # selT: col j of slice [:,11-h:23-h] is 1 iff j==h (den rows)
            selT = const.tile([128, 23], BF16)
            nc.sync.dma_start(selT[:], d_selt.ap())
            # selC: [12, NP*128] f32, col 128*hp+m -> one-hot row 2hp+(m>=64)
            selC = const.tile([12, NP * 128], BF16)
            nc.sync.dma_start(selC[:], d_selc.ap())

            # Per-core data resident in SBUF
            x_fm = res.tile([128, NK, TOK], F32)
            nc.sync.dma_start(x_fm[:], d_x0.ap())
            mask_t = res.tile([128, 8 * 128], BF16)
            nc.sync.dma_start(mask_t[:], d_mask.ap())
            prm = res.tile([128, L * 78 + 12], F32)
            nc.sync.dma_start(prm[:], d_prm.ap())

            def pc(off):   # [128,1] AP at packed-param column `off`
                return prm[:, off:off + 1]

            h_bf = res.tile([128, NK, TOK], BF16)
            qkv_sb = res.tile([128, 12, TOK], BF16)   # Q pairs 0-5, K pairs 6-11
            v_stage = res.tile([128, 2, D], BF16)     # V^T per own tok chunk
            k_all = res.tile([128, NP, 8 * 128], BF16)
            v_all = res.tile([128, 8, D], BF16)       # [tok, key slot, head*64]
            vals_f = res.tile([128, NP, TOK], F32)    # unnormalized attn vals
            vals_fm = res.tile([128, NP, TOK], BF16)
            h2_sb = res.tile([128, NM_HID, TOK], BF16)
            f2part = res.tile([128, NK, TOK], F32)

            with ExitStack() as lctx:
                wpool = lctx.enter_context(tc.tile_pool(name="wpool", bufs=1))
                tmp = lctx.enter_context(tc.tile_pool(name="tmp", bufs=3))
                stt = lctx.enter_context(tc.tile_pool(name="stt", bufs=2))
                psb = lctx.enter_context(tc.tile_pool(name="psb", bufs=2))
                ps_mm = lctx.enter_context(tc.tile_pool(name="ps_mm", bufs=2, space="PSUM"))
                ps_sc = lctx.enter_context(tc.tile_pool(name="ps_sc", bufs=4, space="PSUM"))
                ps_av = lctx.enter_context(tc.tile_pool(name="ps_av", bufs=1, space="PSUM"))
                ps_dn = lctx.enter_context(tc.tile_pool(name="ps_dn", bufs=1, space="PSUM"))

                pools = dict(ps_sc=ps_sc, ps_mm=ps_mm, tmp=tmp, stt=stt,
                             ones_bf=ones_bf, ones_row=ones_row)

                for l in range(L):
                    po = 78 * l
                    _ln(nc, pools, x_fm,
                        [pc(po + 54 + k) for k in range(NK)],
                        [pc(po + 60 + k) for k in range(NK)], h_bf)

                    wq = wpool.tile([128, NK, 3 * D], BF16, tag="wqkv")
                    nc.sync.dma_start(wq[:], d_qkvw.ap()[l])

                    # K first (qkv out cols 768..1536 -> qkv_sb slots 6-11)
                    for m in range(6):
                        ps = ps_mm.tile([128, TOK], F32, tag="mm")
                        for k in range(NK):
                            nc.tensor.matmul(
                                ps[:], wq[:, k, D + 128 * m:D + 128 * (m + 1)],
                                h_bf[:, k, :], start=(k == 0), stop=(k == NK - 1))
                        nc.scalar.activation(qkv_sb[:, 6 + m, :], ps[:], AF.Identity,
                                             bias=pc(po + 6 + m))

                    # V^T: activations stationary, V weight cols streamed.
                    for t in range(2):
                        for hf in range(2):
                            ps = ps_mm.tile([128, 384], F32, tag="mm")
                            for k in range(NK):
                                nc.tensor.matmul(
                                    ps[:], h_bf[:, k, 128 * t:128 * (t + 1)],
                                    wq[:, k, 2 * D + 384 * hf:2 * D + 384 * (hf + 1)],
                                    start=(k == 0), stop=(k == NK - 1))
                            nc.scalar.copy(
                                v_stage[:, t, 384 * hf:384 * (hf + 1)], ps[:])

                    # K exchange first (scores depend only on K)
                    bk_in = dram.tile([6, 128, 256], BF16, tag="bkin")
                    bk_out = dram.tile([4, 6, 128, 256], BF16, tag="bkout")
                    for k in range(6):
                        nc.sync.dma_start(bk_in[k], qkv_sb[:, 6 + k, :])
                    nc.gpsimd.collective_compute(
                        "AllGather", ALU.bypass, replica_groups=GROUPS,
                        ins=[bk_in.opt()], outs=[bk_out.opt()])
                    bv_in = dram.tile([6, 128, 256], BF16, tag="bvin")
                    bv_out = dram.tile([4, 6, 128, 256], BF16, tag="bvout")
                    for t in range(2):
                        for j in range(3):
                            nc.sync.dma_start(bv_in[3 * t + j],
                                              v_stage[:, t, 256 * j:256 * (j + 1)])
                    nc.gpsimd.collective_compute(
                        "AllGather", ALU.bypass, replica_groups=GROUPS,
                        ins=[bv_in.opt()], outs=[bv_out.opt()])

                    # Q while the AllGather flies (cols 0..768 -> slots 0-5)
                    for m in range(6):
                        ps = ps_mm.tile([128, TOK], F32, tag="mm")
                        for k in range(NK):
                            nc.tensor.matmul(
                                ps[:], wq[:, k, 128 * m:128 * (m + 1)],
                                h_bf[:, k, :], start=(k == 0), stop=(k == NK - 1))
                        nc.scalar.activation(qkv_sb[:, m, :], ps[:], AF.Identity,
                                             bias=pc(po + m))

                    # prefetch next-phase weights during the collective
                    wp = wpool.tile([128, NK, D], BF16, tag="wproj")
                    nc.sync.dma_start(wp[:], d_projw.ap()[l])

                    # unpack: key slot 4t+r <- rank r's chunk t
                    for t in range(2):
                        for r in range(4):
                            for k in range(6):
                                nc.sync.dma_start(
                                    k_all[:, k, 128 * (4 * t + r):128 * (4 * t + r + 1)],
                                    bk_out[r, k, :, 128 * t:128 * (t + 1)])
                    for t in range(2):
                        for r in range(4):
                            for j in range(3):
                                nc.sync.dma_start(
                                    v_all[:, 4 * t + r, 256 * j:256 * (j + 1)],
                                    bv_out[r, 3 * t + j])

                    # attention: A = q cols 0:128 (slots 0-3),
                    #            B = q cols 128:256 (slots 0-7)
                    den_all = ps_dn.tile([12, TOK], F32, tag="dn")
                    for hp in range(NP):
                        pA = [None, None]
                        pB = [None, None]
                        for h01 in range(2):
                            o = 64 * h01
                            qsl = slice(o, o + 64)
                            stA = ps_sc.tile([128, 512], F32, tag="sc")
                            for s4 in range(4):
                                nc.tensor.matmul(
                                    stA[:, 128 * s4:128 * (s4 + 1)],
                                    k_all[qsl, hp, 128 * s4:128 * (s4 + 1)],
                                    qkv_sb[qsl, hp, 0:128],
                                    start=True, stop=True)
                            pA[h01] = psb.tile([128, 512], BF16, tag=f"pA{h01}", name=f"pA_{hp}_{h01}")
                            nc.scalar.activation(pA[h01][:], stA[:], AF.Exp)
                            nc.vector.tensor_mul(pA[h01][:], pA[h01][:],
                                                 mask_t[:, 0:512])
                            stB1 = ps_sc.tile([128, 512], F32, tag="sc")
                            stB2 = ps_sc.tile([128, 512], F32, tag="sc")
                            for s8 in range(8):
                                dst = stB1 if s8 < 4 else stB2
                                nc.tensor.matmul(
                                    dst[:, 128 * (s8 % 4):128 * (s8 % 4 + 1)],
                                    k_all[qsl, hp, 128 * s8:128 * (s8 + 1)],
                                    qkv_sb[qsl, hp, 128:256],
                                    start=True, stop=True)
                            pB[h01] = psb.tile([128, 1024], BF16, tag=f"pB{h01}", name=f"pB_{hp}_{h01}")
                            nc.scalar.activation(pB[h01][:, 0:512], stB1[:], AF.Exp)
                            nc.scalar.activation(pB[h01][:, 512:1024], stB2[:], AF.Exp)
                            nc.vector.tensor_mul(pB[h01][:, 512:1024],
                                                 pB[h01][:, 512:1024],
                                                 mask_t[:, 512:1024])

                        # AV (col-tiled head pair) + denominators
                        for cab in range(2):        # 0 = A, 1 = B
                            nsl = 4 if cab == 0 else 8
                            qq = slice(128 * cab, 128 * (cab + 1))
                            av2 = ps_av.tile([128, 128], F32, tag="av")
                            for h01 in range(2):
                                hh = 2 * hp + h01
                                p_t = (pA if cab == 0 else pB)[h01]
                                for s in range(nsl):
                                    nc.tensor.matmul(
                                        av2[64 * h01:64 * (h01 + 1), :],
                                        v_all[:, s, 64 * hh:64 * (hh + 1)],
                                        p_t[:, 128 * s:128 * (s + 1)],
                                        start=(s == 0), stop=(s == nsl - 1))
                            # dens: one global accumulation group in den_all;
                            # each MM adds head hh's sums (zeros elsewhere)
                            for h01 in range(2):
                                hh = 2 * hp + h01
                                p_t = (pA if cab == 0 else pB)[h01]
                                for s in range(nsl):
                                    first = (hp == 0 and cab == 0
                                             and h01 == 0 and s == 0)
                                    last = (hp == NP - 1 and cab == 1
                                            and h01 == 1 and s == nsl - 1)
                                    nc.tensor.matmul(
                                        den_all[:, qq],
                                        selT[:, 11 - hh:23 - hh],
                                        p_t[:, 128 * s:128 * (s + 1)],
                                        start=first, stop=last)
                            nc.vector.tensor_copy(vals_f[:, hp, qq], av2[:])

                    # batched reciprocal 1/x = exp(-ln(x)), then normalize
                    rden12 = stt.tile([12, TOK], BF16, tag="rden")
                    with nc.allow_low_precision(reason="softmax 1/den in bf16"):
                        nc.vector.reciprocal(rden12[:], den_all[:])
                    for hp in range(NP):
                        bc = ps_mm.tile([128, TOK], F32, tag="mm")
                        nc.tensor.matmul(bc[:], selC[:, 128 * hp:128 * (hp + 1)],
                                         rden12[:], start=True, stop=True)
                        nc.vector.tensor_mul(vals_fm[:, hp, :],
                                             vals_f[:, hp, :], bc[:])
                        nc.vector.tensor_scalar_add(vals_fm[:, hp, :],
                                                    vals_fm[:, hp, :],
                                                    pc(po + 12 + hp))

                    # proj + residual
                    for m in range(NK):
                        ps = ps_mm.tile([128, TOK], F32, tag="mm")
                        for k in range(NK):
                            nc.tensor.matmul(ps[:], wp[:, k, 128 * m:128 * (m + 1)],
                                             vals_fm[:, k, :],
                                             start=(k == 0), stop=(k == NK - 1))
                        t2 = tmp.tile([128, TOK], F32, tag="lnt")
                        nc.scalar.activation(t2[:], ps[:], AF.Identity,
                                             bias=pc(po + 18 + m))
                        nc.vector.tensor_add(x_fm[:, m, :], x_fm[:, m, :], t2[:])

                    # LN2 + FFN
                    _ln(nc, pools, x_fm,
                        [pc(po + 66 + k) for k in range(NK)],
                        [pc(po + 72 + k) for k in range(NK)], h_bf)

                    for hf in range(2):
                        w1 = wpool.tile([128, NK, HID // 2], BF16, tag="wf1",
                                        name=f"w1_{l}_{hf}")
                        nc.sync.dma_start(
                            w1[:], d_f1w.ap()[l][:, :, 1536 * hf:1536 * (hf + 1)])
                        for m in range(12):
                            mm = 12 * hf + m
                            ps = ps_mm.tile([128, TOK], F32, tag="mm")
                            for k in range(NK):
                                nc.tensor.matmul(
                                    ps[:], w1[:, k, 128 * m:128 * (m + 1)],
                                    h_bf[:, k, :],
                                    start=(k == 0), stop=(k == NK - 1))
                            nc.scalar.activation(h2_sb[:, mm, :], ps[:], AF.Gelu,
                                                 bias=pc(po + 24 + mm))

                    for hf in range(2):
                        w2 = wpool.tile([128, 12, D], BF16, tag="wf2",
                                        name=f"w2_{l}_{hf}")
                        nc.sync.dma_start(
                            w2[:], d_f2w.ap()[l][:, 12 * hf:12 * (hf + 1), :])
                        for m in range(NK):
                            ps = ps_mm.tile([128, TOK], F32, tag="mm")
                            for k in range(12):
                                nc.tensor.matmul(
                                    ps[:], w2[:, k, 128 * m:128 * (m + 1)],
                                    h2_sb[:, 12 * hf + k, :],
                                    start=(k == 0), stop=(k == 11))
                            if hf == 0:
                                nc.vector.tensor_copy(f2part[:, m, :], ps[:])
                            else:
                                nc.vector.tensor_add(f2part[:, m, :],
                                                     f2part[:, m, :], ps[:])
                                t2 = tmp.tile([128, TOK], F32, tag="lnt")
                                nc.scalar.activation(t2[:], f2part[:, m, :],
                                                     AF.Identity,
                                                     bias=pc(po + 48 + m))
                                nc.vector.tensor_add(x_fm[:, m, :],
                                                     x_fm[:, m, :], t2[:])

                # final LN
                _ln(nc, pools, x_fm,
                    [pc(78 * L + k) for k in range(NK)],
                    [pc(78 * L + 6 + k) for k in range(NK)], h_bf)

            # ---- all-gather final hiddens across all 8 cores ----
            b2_in = dram.tile([NK, 128, TOK], BF16, tag="b2in")
            b2_out = dram.tile([N_CORES, NK, 128, TOK], BF16, tag="b2out",
                                addr_space="Shared")
            for k in range(NK):
                nc.sync.dma_start(b2_in[k], h_bf[:, k, :])
            nc.gpsimd.collective_compute(
                "AllGather", ALU.bypass, replica_groups=ALL_GROUP,
                ins=[b2_in.opt()], outs=[b2_out.opt()])

            with ExitStack() as mctx:
                lmw = mctx.enter_context(tc.tile_pool(name="lmw", bufs=2))
                lmo = mctx.enter_context(tc.tile_pool(name="lmo", bufs=3))
                lmh = mctx.enter_context(tc.tile_pool(name="lmh", bufs=1))
                ps_lm = mctx.enter_context(tc.tile_pool(name="ps_lm", bufs=8, space="PSUM"))

                h_all = lmh.tile([128, NK, NQT * 128], BF16)
                for r in range(N_CORES):
                    for k in range(NK):
                        nc.sync.dma_start(h_all[:, k, TOK * r:TOK * (r + 1)],
                                          b2_out[r, k])

                # lm_head: 2 groups of 4 vocab blocks; LDW amortized 4x
                for g2 in range(2):
                    wv = lmw.tile([128, NK, 4 * VB], BF16, tag="wv")
                    for j in range(4):
                        nc.sync.dma_start(wv[:, :, VB * j:VB * (j + 1)],
                                          d_wemb.ap()[4 * g2 + j])
                    for qt in range(NQT):
                        pss = [ps_lm.tile([128, VB], F32, tag="lmps",
                                          name=f"lmps_{g2}_{qt}_{j}")
                               for j in range(4)]
                        for k in range(NK):
                            lhs = h_all[:, k, 128 * qt:128 * (qt + 1)]
                            for j in range(4):
                                nc.tensor.matmul(pss[j][:], lhs,
                                                 wv[:, k, VB * j:VB * (j + 1)],
                                                 start=(k == 0), stop=(k == NK - 1))
                        ot = lmo.tile([128, 4 * VB], BF16, tag="ot")
                        nc.vector.tensor_copy(ot[:, 0:VB], pss[0][:])
                        nc.vector.tensor_copy(ot[:, VB:2 * VB], pss[1][:])
                        nc.scalar.copy(ot[:, 2 * VB:3 * VB], pss[2][:])
                        nc.scalar.copy(ot[:, 3 * VB:4 * VB], pss[3][:])
                        nc.sync.dma_start(
                            d_out.ap()[qt][:, 2000 * g2:2000 * (g2 + 1)], ot[:])

    nc.compile()
    return nc


def _prep_inputs(W_emb, pos_emb, norm1_g, norm1_b, qkv_w, qkv_b, proj_w, proj_b,
                 norm2_g, norm2_b, ffn_w1, ffn_b1, ffn_w2, ffn_b2, fin_g, fin_b,
                 input_ids):
    bf = ml_dtypes.bfloat16
    f32 = np.float32

    def tp(a):  # [L, out, in] -> [L, 128, NK, out] bf16 (partition-major)
        a = np.asarray(a, f32)
        out_dim = a.shape[1]
        return np.ascontiguousarray(
            a.transpose(0, 2, 1).reshape(L, NK, 128, out_dim)
            .transpose(0, 2, 1, 3)).astype(bf)

    # qkv reorder [L,H,3,HD,D] -> [L,3,H,HD,D]; q pre-scaled by 1/sqrt(HD)
    qkv_r = np.asarray(qkv_w, f32).reshape(L, H, 3, HD, D) \
        .transpose(0, 2, 1, 3, 4).copy()
    qkv_r[:, 0] *= 1.0 / math.sqrt(HD)
    qkv_r = qkv_r.reshape(L, 3 * D, D)
    qkv_b_r = np.asarray(qkv_b, f32).reshape(L, H, 3, HD) \
        .transpose(0, 2, 1, 3).copy()
    qkv_b_r[:, 0] *= 1.0 / math.sqrt(HD)
    qkv_b_r = qkv_b_r.reshape(L, 3 * D)

    f2w_t = np.ascontiguousarray(
        np.asarray(ffn_w2, f32).transpose(0, 2, 1)
        .reshape(L, NM_HID, 128, D).transpose(0, 2, 1, 3)).astype(bf)

    def btile(a, nm):  # [L, nm*128] -> [L, 128, nm]
        return np.asarray(a, f32).reshape(L, nm, 128).transpose(0, 2, 1)

    prm = np.zeros((128, L * 78 + 12), f32)
    packs = [(0, btile(qkv_b_r, 18)),
             (18, btile(proj_b, NK)), (24, btile(ffn_b1, NM_HID)),
             (48, btile(ffn_b2, NK)), (54, btile(norm1_g, NK)),
             (60, btile(norm1_b, NK)), (66, btile(norm2_g, NK)),
             (72, btile(norm2_b, NK))]
    for l in range(L):
        for off, arr in packs:
            prm[:, 78 * l + off:78 * l + off + arr.shape[2]] = arr[l]
    prm[:, 78 * L:78 * L + 6] = np.asarray(fin_g, f32).reshape(NK, 128).T
    prm[:, 78 * L + 6:78 * L + 12] = np.asarray(fin_b, f32).reshape(NK, 128).T

    W_emb = np.asarray(W_emb, f32)
    wemb_slices = []
    for c in range(N_CORES):
        sl = W_emb[VSH * c:VSH * (c + 1)].T  # [768, 4000]
        wemb_slices.append(np.ascontiguousarray(
            sl.reshape(NK, 128, NVB, VB).transpose(2, 1, 0, 3)).astype(bf))

    ids = np.asarray(input_ids).reshape(B, S).astype(np.int64)
    pos = np.asarray(pos_emb, f32)

    selt = np.zeros((128, 23), f32)
    selt[:, 11] = 1.0
    selc = np.zeros((12, NP * 128), f32)
    for hp in range(NP):
        selc[2 * hp, 128 * hp:128 * hp + 64] = 1.0
        selc[2 * hp + 1, 128 * hp + 64:128 * (hp + 1)] = 1.0

    common = {
        "qkvw": tp(qkv_r), "projw": tp(np.asarray(proj_w, f32)),
        "f1w": tp(np.asarray(ffn_w1, f32)), "f2w": f2w_t,
        "prm": prm, "selt": selt.astype(bf), "selc": selc.astype(bf),
    }

    in_maps = []
    for c in range(N_CORES):
        g, p = c // 4, c % 4
        ca, cb = p, 7 - p
        tok_idx = np.concatenate([np.arange(128 * ca, 128 * (ca + 1)),
                                  np.arange(128 * cb, 128 * (cb + 1))])
        x0 = W_emb[ids[g, tok_idx]] * math.sqrt(D) + pos[tok_idx]
        xs = np.ascontiguousarray(
            x0.T.reshape(NK, 128, TOK).transpose(1, 0, 2)).astype(f32)
        # masks: slots 0-3 (key chunks 0-3) vs chunk A; slots 4-7
        # (key chunk 7-r at slot 4+r) vs chunk B
        m8 = np.zeros((8, 128, 128), f32)
        qa_pos = 128 * ca + np.arange(128)
        qb_pos = 128 * cb + np.arange(128)
        for s4 in range(4):
            kpos = 128 * s4 + np.arange(128)
            m8[s4] = (kpos[:, None] <= qa_pos[None, :])
        for r in range(4):
            kpos = 128 * (7 - r) + np.arange(128)
            m8[4 + r] = (kpos[:, None] <= qb_pos[None, :])
        in_maps.append({"x0": xs,
                        "mask": np.ascontiguousarray(
                            m8.transpose(1, 0, 2).reshape(128, 8 * 128)).astype(bf),
                        "wemb": wemb_slices[c], **common})
    return in_maps


def kernel(**inputs):
    global LAST_RESULT, _NC_CACHE
    in_maps = _prep_inputs(**inputs)
    if _NC_CACHE is None:
        _NC_CACHE = build_nc()
    res = run_bass_kernel_spmd(_NC_CACHE, in_maps, list(range(N_CORES)),
                               trace=TRACE)
    LAST_RESULT = res
    out = np.zeros((B * S, V), np.float32)
    for c in range(N_CORES):
        o = np.asarray(res.results[c]["logits"]).astype(np.float32)
        for qt in range(NQT):
            r, t = qt // 2, qt % 2
            g, p = r // 4, r % 4
            chunk = 8 * g + (p if t == 0 else 7 - p)
            out[128 * chunk:128 * (chunk + 1), VSH * c:VSH * (c + 1)] = o[qt]
    return out.reshape(B, S, V)
